# revision 1
# baseline (speedup 1.0000x reference)
"""Multi-head attention (B=2, N=2048, D=1024, H=16) on 8 TRN2 NeuronCores.

Sharding: tensor-parallel over heads across all 8 cores. Core i handles
heads [2i, 2i+2) (128 of the 1024 QKV output dims) for BOTH batches.
After local attention, an 8-core AllToAll (split in two, one per local
head, so the first overlaps the second head's attention) re-shards from
head-split to (batch, sequence-slab)-split; core j then computes the
output projection for batch j//4, rows [512*(j%4), 512*(j%4+1)).
Host-side work is slicing/layout only (x is passed transposed [D, N] per
batch in bf16 — the shard layout the device consumes directly).

Device layout notes:
  - Q^T, K^T [d, q] come straight from matmuls of weight chunks against
    xT; V is built in [k, d] form with xT chunks as the stationary side.
  - Scores are computed transposed (S^T [k, q]) so exp(S^T) tiles feed the
    AV matmul as the moving operand with k on partitions.
  - V gets an appended ones column per head, so the AV matmul also emits
    softmax denominators (row 64 of the [65, q] accumulator) for free.
  - All matmuls run in bf16 with fp32 PSUM accumulation (8.0e-3 rel err
    end to end vs the f32 reference; the gate is 2e-2).
"""

import ml_dtypes
import numpy as np

import concourse.bass as bass
import concourse.mybir as mybir
import concourse.tile as tile
from concourse import bacc
from concourse.bass_utils import run_bass_kernel_spmd
from concourse.masks import make_identity

F32 = mybir.dt.float32
BF16 = mybir.dt.bfloat16
EXP = mybir.ActivationFunctionType.Exp
IDENT = mybir.ActivationFunctionType.Identity
BYPASS = mybir.AluOpType.bypass

P = 128
B, N, D = 2, 2048, 1024
NH, HD = 16, 64
TP = 8                  # head-parallel group size (all cores)
HL = D // TP            # 128 local head dims (2 heads x 64)
NHL = NH // TP          # 2 local heads
QS = 512                # query slab width
NQS = N // QS           # 4 slabs
KC = N // P             # 16 key chunks of 128
DC = D // P             # 8 model-dim chunks of 128
ROWS = 512              # output rows per core (one slab of one batch)
RG = [[0, 1, 2, 3, 4, 5, 6, 7]]
SCALE = 1.0 / np.sqrt(HD)

_CACHE = {}


def build_nc(reps=1):
    nc = bacc.Bacc("TRN2", target_bir_lowering=False, debug=False,
                   num_devices=8)

    xt_ext = nc.declare_dram_parameter("xt", [B, D, N], BF16, isOutput=False)
    wq_ext = nc.declare_dram_parameter("wq", [P, DC, HL], BF16, isOutput=False)
    bq_ext = nc.declare_dram_parameter("bq", [P, 1], F32, isOutput=False)
    wk_ext = nc.declare_dram_parameter("wk", [P, DC, HL], BF16, isOutput=False)
    bk_ext = nc.declare_dram_parameter("bk", [P, 1], F32, isOutput=False)
    wv_ext = nc.declare_dram_parameter("wv", [P, DC, HL], BF16, isOutput=False)
    bv_ext = nc.declare_dram_parameter("bv", [1, HL], F32, isOutput=False)
    wo_ext = nc.declare_dram_parameter("wo", [P, DC, D], BF16, isOutput=False)
    bo_ext = nc.declare_dram_parameter("bo", [1, D], F32, isOutput=False)
    out_ext = nc.declare_dram_parameter("out", [ROWS, D], F32, isOutput=True)

    with tile.TileContext(nc) as tc:
        with (
            tc.tile_pool(name="const", bufs=1) as const,
            tc.tile_pool(name="persist", bufs=1) as persist,
            tc.tile_pool(name="dram", bufs=1, space="DRAM") as dram,
            tc.tile_pool(name="xtp", bufs=2) as xtp,
            tc.tile_pool(name="wp", bufs=1) as wp,
            tc.tile_pool(name="vtp", bufs=3) as vtp,
            tc.tile_pool(name="wo_p", bufs=1) as wo_p,
            tc.tile_pool(name="ptp", bufs=6) as ptp,
            tc.tile_pool(name="nrm", bufs=4) as nrm,
            tc.tile_pool(name="psA", bufs=3, space="PSUM") as psA,
            tc.tile_pool(name="psB", bufs=2, space="PSUM") as psB,
        ):
            identity_b = const.tile([P, P], BF16)
            make_identity(nc, identity_b)

            # persistent SBUF tensors
            QT = persist.tile([P, B, N], BF16)        # [128 d, b, 2048 q]
            KT = persist.tile([P, B, N], BF16)        # [128 d, b, 2048 k]
            Vaug = persist.tile([P, B, KC, NHL, HD + 1], BF16)
            nc.gpsimd.memset(Vaug[:, :, :, :, HD:HD + 1], 1.0)

            a2a_in = [dram.tile([TP, HD, QS], BF16, name=f"a2a_in{h}")
                      for h in range(NHL)]
            a2a_out = [dram.tile([TP, HD, QS], BF16, name=f"a2a_out{h}")
                       for h in range(NHL)]

            wq_sb = wp.tile([P, DC, HL], BF16)
            wk_sb = wp.tile([P, DC, HL], BF16)
            wv_sb = wp.tile([P, DC, HL], BF16)
            for w_sb, w_ext in ((wq_sb, wq_ext), (wk_sb, wk_ext),
                                (wv_sb, wv_ext)):
                nc.sync.dma_start(w_sb, w_ext[:])

            bqs = wp.tile([P, 1], F32)   # pre-scaled by 1/sqrt(HD) on host
            bks = wp.tile([P, 1], F32)
            nc.sync.dma_start(bqs, bq_ext[:])
            nc.sync.dma_start(bks, bk_ext[:])
            bv_sb = wp.tile([1, HL], F32)
            nc.sync.dma_start(bv_sb, bv_ext[:])
            bv_bc = wp.tile([P, HL], F32)
            nc.gpsimd.partition_broadcast(bv_bc[:], bv_sb[:])


            def qkv(b):
                xT = xtp.tile([P, DC, N], BF16, tag="xT", name=f"xT{b}")
                # chunked so the first matmuls start when D-chunk 0 lands
                for dc in range(DC):
                    for qh in range(2):
                        nc.sync.dma_start(
                            xT[:, dc, qh * (N // 2):(qh + 1) * (N // 2)],
                            xt_ext[b, dc * P:(dc + 1) * P,
                                   qh * (N // 2):(qh + 1) * (N // 2)])

                # Q^T, K^T : [128 d, 2048], d on partitions
                for w_sb, bias, scl, dst in (
                    (wq_sb, bqs, SCALE, QT),
                    (wk_sb, bks, 1.0, KT),
                ):
                    for qs in range(NQS):
                        psm = psA.tile([P, QS], F32, tag="pss", name="psm")
                        for dc in range(DC):
                            nc.tensor.matmul(
                                psm,
                                lhsT=w_sb[:, dc, :],
                                rhs=xT[:, dc, qs * QS:(qs + 1) * QS],
                                start=(dc == 0), stop=(dc == DC - 1))
                        nc.scalar.activation(
                            dst[:, b, qs * QS:(qs + 1) * QS], psm,
                            IDENT, bias=bias[:, 0:1], scale=scl)

                # V^T : [128 d, 2048 k] (N=512 matmuls), then PE-transpose
                # 128x128 chunks into Vaug's [k, d] form
                for ks in range(NQS):
                    psm = psA.tile([P, QS], F32, tag="pss", name="psm")
                    for dc in range(DC):
                        nc.tensor.matmul(
                            psm,
                            lhsT=wv_sb[:, dc, :],
                            rhs=xT[:, dc, ks * QS:(ks + 1) * QS],
                            start=(dc == 0), stop=(dc == DC - 1))
                    vt_t = vtp.tile([P, QS], BF16, name="vt_t")
                    nc.vector.tensor_copy(vt_t, psm)
                    for kk in range(QS // P):
                        kc = ks * (QS // P) + kk
                        pst = psB.tile([P, P], BF16, tag="acc", name="pst")
                        nc.tensor.transpose(
                            pst, vt_t[:, kk * P:(kk + 1) * P], identity_b)
                        nc.vector.tensor_add(
                            out=Vaug[:, b, kc, :, :HD],
                            in0=pst[:].rearrange("p (h d) -> p h d", d=HD),
                            in1=bv_bc[:].rearrange("p (h d) -> p h d", d=HD))

            def attn(h, b):
                po = h * HD
                for qs in range(NQS):
                    j = b * NQS + qs      # a2a destination core
                    acc = psB.tile([P, QS], F32, tag="acc", name="acc")
                    for kc2 in range(KC // 2):
                        # two score chunks into one 2-bank PSUM tile so a
                        # single exp covers both
                        pss = psA.tile([P, 2 * QS], F32, tag="pss", name="pss")
                        for hf in range(2):
                            kc = 2 * kc2 + hf
                            nc.tensor.matmul(
                                pss[:, hf * QS:(hf + 1) * QS],
                                lhsT=KT[po:po + HD, b, kc * P:(kc + 1) * P],
                                rhs=QT[po:po + HD, b, qs * QS:(qs + 1) * QS],
                                start=True, stop=True)
                        pt = ptp.tile([P, 2 * QS], BF16, name="pt")
                        nc.scalar.activation(pt, pss, EXP)
                        for hf in range(2):
                            kc = 2 * kc2 + hf
                            nc.tensor.matmul(
                                acc[:HD + 1],
                                lhsT=Vaug[:, b, kc, h, :],
                                rhs=pt[:, hf * QS:(hf + 1) * QS],
                                start=(kc == 0), stop=(kc == KC - 1))
                    rec = nrm.tile([1, QS], BF16, name="rec")
                    with nc.allow_low_precision(
                            reason="softmax denom reciprocal to bf16"):
                        nc.vector.reciprocal(rec, acc[HD:HD + 1])
                    bc_sb = nrm.tile([HD, QS], BF16, tag="bcsb", name="bc_sb")
                    nc.gpsimd.partition_broadcast(bc_sb[:], rec[:])
                    onrm = nrm.tile([HD, QS], BF16, tag="onrm", name="onrm")
                    nc.vector.tensor_mul(onrm, acc[:HD], bc_sb)
                    nc.sync.dma_start(a2a_in[h][j, :, :], onrm)

            for _rep in range(reps):
                qkv(0)
                attn(0, 0)
                qkv(1)
                attn(0, 1)
                nc.gpsimd.collective_compute(
                    "AllToAll", BYPASS,
                    ins=[a2a_in[0][:].opt()],
                    outs=[a2a_out[0][:].opt()],
                    replica_groups=RG)
                # load wo late so it doesn't compete with xT DMA at start
                wo_sb = wo_p.tile([P, DC, D], BF16, tag="wo_sb", name="wo_sb")
                nc.sync.dma_start(wo_sb, wo_ext[:])
                bo_sb = wo_p.tile([1, D], F32, tag="bo_sb", name="bo_sb")
                nc.sync.dma_start(bo_sb, bo_ext[:])
                bo_bc = wo_p.tile([P, D], F32, tag="bo_bc", name="bo_bc")
                nc.gpsimd.partition_broadcast(bo_bc[:], bo_sb[:])
                attn(1, 0)
                attn(1, 1)
                nc.gpsimd.collective_compute(
                    "AllToAll", BYPASS,
                    ins=[a2a_in[1][:].opt()],
                    outs=[a2a_out[1][:].opt()],
                    replica_groups=RG)

                # ---------------- output projection ----------------
                # ot_sb partitions: p = h*64+d within each source core's 128
                ot_sb = wo_p.tile([P, DC, QS], BF16, name="ot_sb")
                for h in range(NHL):
                    # per-source-core chunks so the first O-proj matmul can
                    # start as soon as src 0's slice lands
                    for s in range(TP):
                        nc.sync.dma_start(
                            ot_sb[h * HD:(h + 1) * HD, s, :],
                            a2a_out[h][s].rearrange("p q -> p q"))
                for mq in range(ROWS // P):
                    for oc in range(2):
                        psm = psA.tile([P, QS], F32, tag="pss", name="psm2")
                        for dc in range(DC):
                            nc.tensor.matmul(
                                psm,
                                lhsT=ot_sb[:, dc, mq * P:(mq + 1) * P],
                                rhs=wo_sb[:, dc, oc * QS:(oc + 1) * QS],
                                start=(dc == 0), stop=(dc == DC - 1))
                        o_t = nrm.tile([P, QS], F32, tag="ot", name="o_t")
                        nc.vector.tensor_add(
                            out=o_t, in0=psm,
                            in1=bo_bc[:, oc * QS:(oc + 1) * QS])
                        nc.sync.dma_start(
                            out_ext[mq * P:(mq + 1) * P,
                                    oc * QS:(oc + 1) * QS], o_t)

    nc.finalize()
    return nc


def _chunked(w):
    # [D, n] -> [P, DC, n]: row r = c*P + p lands at [p, c]
    n = w.shape[1]
    return np.ascontiguousarray(w.reshape(DC, P, n).transpose(1, 0, 2))


def make_in_maps(inputs):
    bf = ml_dtypes.bfloat16
    x = np.asarray(inputs["x"], dtype=np.float32)
    # host-side shard layout: x transposed per batch, bf16; weights in the
    # [partition, chunk, col] layout SBUF consumes (contiguous DMAs)
    xt = np.ascontiguousarray(x.transpose(0, 2, 1)).astype(bf)
    full_w = {k: np.asarray(inputs[k], np.float32).astype(bf)
              for k in ("wq", "wk", "wv", "wo")}
    full_b = {k: np.asarray(inputs[k], np.float32)
              for k in ("bq", "bk", "bv", "bo")}
    bq_scaled = (full_b["bq"] * SCALE).astype(np.float32)
    wo_r = _chunked(full_w["wo"])
    bo_r = full_b["bo"].reshape(1, D)
    in_maps = []
    for i in range(8):
        hs = i * HL
        m = {"xt": xt,
             "wq": _chunked(full_w["wq"][:, hs:hs + HL]),
             "wk": _chunked(full_w["wk"][:, hs:hs + HL]),
             "wv": _chunked(full_w["wv"][:, hs:hs + HL]),
             "bq": np.ascontiguousarray(bq_scaled[hs:hs + HL].reshape(1, P).T),
             "bk": np.ascontiguousarray(full_b["bk"][hs:hs + HL].reshape(1, P).T),
             "bv": full_b["bv"][hs:hs + HL].reshape(1, HL),
             "wo": wo_r,
             "bo": bo_r}
        in_maps.append(m)
    return in_maps


def kernel(**inputs):
    if "nc" not in _CACHE:
        _CACHE["nc"] = build_nc()
    nc = _CACHE["nc"]
    in_maps = make_in_maps(inputs)
    res = run_bass_kernel_spmd(nc, in_maps, core_ids=list(range(8)))
    out = np.empty((B, N, D), dtype=np.float32)
    for j in range(8):
        b, t = j // NQS, j % NQS
        out[b, t * ROWS:(t + 1) * ROWS] = res.results[j]["out"]
    return out

